# revision 2
# baseline (speedup 1.0000x reference)
"""Trainium2 Bass kernel for nn_Attention_36146444763783.

GroupNorm(32) + SiLU -> QKV proj -> 8-head attention (n=1024) -> out proj
+ bias + residual, batch=16, fully data-parallel: 2 batches per NeuronCore
across 8 cores.

Per-core dataflow (all matmuls bf16 with fp32 PSUM accumulation):
  - x [2,1024,512] fp32 loaded as [128, 8*512] tiles (partition = token%128)
  - GroupNorm stats per (batch, group) via DVE reduces + PE ones-matmul
    partition sums; per-channel affine A,B expanded to [128,4] via a
    selector matmul so the normalize runs as one scalar-engine
    activation (Silu) per transposed block: xnT = silu(x^T * A + B)
  - QKV: q,k as [d, n] (w stationary), v as [n, d] (xnT stationary),
    with q pre-scaled by 1/8 (folded into w on host)
  - attention per head: simT[j,i] = k^T q on PE; exp on ScalarE
    (no max-subtract: |sim| < 1 for this distribution); PV accumulates
    attn-out [i, d] with an extra all-ones V column producing sumexp[i]
    on the same partitions, normalized during the PSUM drain
  - out proj from PE-transposed attn-out, residual + bias added on DVE
"""

import os
import sys

import numpy as np

sys.path.insert(0, "/opt/trn_rl_repo")

B, HGT, WID, CH = 16, 32, 32, 512
HEADS, HEAD_CH, HIDDEN = 8, 64, 512
GROUPS = 32
EPS = 1e-5
N = HGT * WID  # 1024 tokens per batch
N_CORES = 8
BPC = B // N_CORES  # batches per core
NT = N // 128  # 8 token tiles
CC = CH // 128  # 4 channel chunks


def build_program():
    import concourse.bacc as bacc
    import concourse.mybir as mybir
    import concourse.tile as tile
    from contextlib import ExitStack

    dt = mybir.dt
    f32, bf16 = dt.float32, dt.bfloat16
    AX = mybir.AxisListType
    AF = mybir.ActivationFunctionType

    nc = bacc.Bacc("TRN2", target_bir_lowering=False, debug=False)

    x_d = nc.dram_tensor("x", [BPC, N, CH], f32, kind="ExternalInput").ap()
    wqkv_d = nc.dram_tensor("wqkv", [CH, 3 * HIDDEN], bf16, kind="ExternalInput").ap()
    wout_d = nc.dram_tensor("wout", [HIDDEN, CH], bf16, kind="ExternalInput").ap()
    identf_d = nc.dram_tensor("identf", [128, 128], f32, kind="ExternalInput").ap()
    identb_d = nc.dram_tensor("identb", [128, 128], bf16, kind="ExternalInput").ap()
    sel32_d = nc.dram_tensor("sel32", [32, 128], f32, kind="ExternalInput").ap()
    mask32_d = nc.dram_tensor("mask32", [32, 4], f32, kind="ExternalInput").ap()
    gns_d = nc.dram_tensor("gns", [128, 4], f32, kind="ExternalInput").ap()
    gno_d = nc.dram_tensor("gno", [128, 4], f32, kind="ExternalInput").ap()
    bb_d = nc.dram_tensor("bb", [128, CH], f32, kind="ExternalInput").ap()
    ones_d = nc.dram_tensor("ones", [128, 1], f32, kind="ExternalInput").ap()
    out_d = nc.dram_tensor("out", [BPC, N, CH], f32, kind="ExternalOutput").ap()

    with ExitStack() as ctx:
        tc = ctx.enter_context(tile.TileContext(nc))
        pc = ctx.enter_context(tc.tile_pool(name="const", bufs=1))
        px = ctx.enter_context(tc.tile_pool(name="px", bufs=2))
        psq = ctx.enter_context(tc.tile_pool(name="psq", bufs=2))
        pst = ctx.enter_context(tc.tile_pool(name="pst", bufs=3))
        ptiny = ctx.enter_context(tc.tile_pool(name="ptiny", bufs=2))
        pxnT = ctx.enter_context(tc.tile_pool(name="pxnT", bufs=6))
        pq = ctx.enter_context(tc.tile_pool(name="pq", bufs=6))
        pk = ctx.enter_context(tc.tile_pool(name="pk", bufs=6))
        pv = ctx.enter_context(tc.tile_pool(name="pv", bufs=10))
        pe = ctx.enter_context(tc.tile_pool(name="pe", bufs=10))
        pao = ctx.enter_context(tc.tile_pool(name="pao", bufs=10))
        paoT = ctx.enter_context(tc.tile_pool(name="paoT", bufs=6))
        prc = ctx.enter_context(tc.tile_pool(name="prc", bufs=6))
        pout = ctx.enter_context(tc.tile_pool(name="pout", bufs=2))
        pps = ctx.enter_context(tc.tile_pool(name="pps", bufs=2, space="PSUM"))
        ppsim = ctx.enter_context(tc.tile_pool(name="ppsim", bufs=2, space="PSUM"))
        pppv = ctx.enter_context(tc.tile_pool(name="pppv", bufs=1, space="PSUM"))
        ppst = ctx.enter_context(tc.tile_pool(name="ppst", bufs=1, space="PSUM"))

        # ---- constants ----
        wqkv = []
        for j in range(CC):
            t = pc.tile([128, 3 * HIDDEN], bf16, name=f"wqkv{j}", tag=f"wqkv{j}")
            nc.sync.dma_start(out=t[:], in_=wqkv_d[128 * j : 128 * (j + 1), :])
            wqkv.append(t)
        wout = []
        for j in range(CC):
            t = pc.tile([128, CH], bf16, name=f"wout{j}", tag=f"wout{j}")
            nc.sync.dma_start(out=t[:], in_=wout_d[128 * j : 128 * (j + 1), :])
            wout.append(t)
        identf = pc.tile([128, 128], f32, name="identf", tag="identf")
        nc.sync.dma_start(out=identf[:], in_=identf_d[:, :])
        identb = pc.tile([128, 128], bf16, name="identb", tag="identb")
        nc.sync.dma_start(out=identb[:], in_=identb_d[:, :])
        sel32 = pc.tile([32, 128], f32, name="sel32", tag="sel32")
        nc.sync.dma_start(out=sel32[:], in_=sel32_d[:, :])
        mask32 = pc.tile([32, 4], f32, name="mask32", tag="mask32")
        nc.sync.dma_start(out=mask32[:], in_=mask32_d[:, :])
        gns = pc.tile([128, 4], f32, name="gns", tag="gns")
        nc.sync.dma_start(out=gns[:], in_=gns_d[:, :])
        gno = pc.tile([128, 4], f32, name="gno", tag="gno")
        nc.sync.dma_start(out=gno[:], in_=gno_d[:, :])
        bb = pc.tile([128, CH], f32, name="bb", tag="bb")
        nc.sync.dma_start(out=bb[:], in_=bb_d[:, :])
        ones = pc.tile([128, 1], f32, name="ones", tag="ones")
        nc.sync.dma_start(out=ones[:], in_=ones_d[:, :])

        for b in range(BPC):
            # ---- load x batch: [128, 8*512], partition = token % 128 ----
            xb = px.tile([128, NT * CH], f32, name=f"xb{b}", tag="x")
            nc.sync.dma_start(
                out=xb[:].rearrange("p (t c) -> p t c", t=NT),
                in_=x_d[b].rearrange("(t p) c -> p t c", p=128),
            )

            # ---- GroupNorm stats ----
            ps_st = ppst.tile([32, 2], f32, name=f"ps_st{b}", tag="stps")
            for nt in range(NT):
                st = pst.tile([128, 64], f32, name=f"st{b}_{nt}", tag="stats")
                xv = xb[:, CH * nt : CH * (nt + 1)].rearrange(
                    "p (g k) -> p g k", g=GROUPS
                )
                nc.vector.reduce_sum(out=st[:, 0:32], in_=xv, axis=AX.X)
                sq = psq.tile([128, CH], f32, name=f"sq{b}_{nt}", tag="sq")
                nc.vector.tensor_mul(
                    sq[:], xb[:, CH * nt : CH * (nt + 1)], xb[:, CH * nt : CH * (nt + 1)]
                )
                nc.vector.reduce_sum(
                    out=st[:, 32:64],
                    in_=sq[:].rearrange("p (g k) -> p g k", g=GROUPS),
                    axis=AX.X,
                )
                nc.tensor.matmul(
                    out=ps_st[:, 0:1], lhsT=st[:, 0:32], rhs=ones[:],
                    start=(nt == 0), stop=False,
                )
                nc.tensor.matmul(
                    out=ps_st[:, 1:2], lhsT=st[:, 32:64], rhs=ones[:],
                    start=False, stop=(nt == NT - 1),
                )

            # ---- group mean/rstd -> per-channel affine A, B [128, 4] ----
            g1 = ptiny.tile([32, 8], f32, name=f"g1{b}", tag="g1")
            inv_n = 1.0 / (N * (CH // GROUPS))
            nc.vector.tensor_scalar_mul(g1[:, 0:1], ps_st[:, 0:1], inv_n)  # mean
            nc.vector.tensor_scalar_mul(g1[:, 1:2], ps_st[:, 1:2], inv_n)  # E[x^2]
            nc.vector.tensor_mul(g1[:, 2:3], g1[:, 0:1], g1[:, 0:1])
            nc.vector.tensor_sub(g1[:, 3:4], g1[:, 1:2], g1[:, 2:3])  # var
            nc.vector.tensor_scalar_add(g1[:, 4:5], g1[:, 3:4], EPS)
            nc.vector.reciprocal(g1[:, 5:6], g1[:, 4:5])
            nc.scalar.activation(g1[:, 6:7], g1[:, 5:6], AF.Sqrt)  # rstd
            selr = ptiny.tile([32, 8], f32, name=f"selr{b}", tag="selr")
            nc.vector.tensor_scalar_mul(selr[:, 0:4], mask32[:], g1[:, 6:7])
            nc.vector.tensor_scalar_mul(selr[:, 4:8], mask32[:], g1[:, 0:1])
            ps_ab = ppst.tile([128, 8], f32, name=f"ps_ab{b}", tag="stps")
            nc.tensor.matmul(out=ps_ab[:], lhsT=sel32[:], rhs=selr[:])
            A = ptiny.tile([128, 4], f32, name=f"A{b}", tag="A")
            Bt = ptiny.tile([128, 4], f32, name=f"Bt{b}", tag="Bt")
            tmb = ptiny.tile([128, 4], f32, name=f"tmb{b}", tag="tmb")
            nc.vector.tensor_mul(A[:], ps_ab[:, 0:4], gns[:])
            nc.vector.tensor_mul(tmb[:], ps_ab[:, 4:8], A[:])
            nc.vector.tensor_sub(Bt[:], gno[:], tmb[:])

            # ---- transposed normalize: xnT[j] = silu(x^T * A + B), bf16 ----
            # silu(u) = u * sigmoid(u); u computed on DVE, sigmoid on ScalarE
            xnT = []
            for j in range(CC):
                t = pxnT.tile([128, N], bf16, name=f"xnT{b}_{j}", tag="xnT")
                xnT.append(t)
            for j in range(CC):
                for half in range(2):
                    pt = pps.tile([128, 512], f32, name=f"pt{b}_{j}_{half}", tag="ps512")
                    for q in range(4):
                        nt = 4 * half + q
                        nc.tensor.matmul(
                            out=pt[:, 128 * q : 128 * (q + 1)],
                            lhsT=xb[:, CH * nt + 128 * j : CH * nt + 128 * (j + 1)],
                            rhs=identf[:],
                            is_transpose=True,
                            start=(q == 0), stop=(q == 3),
                        )
                    u = ptiny.tile([128, 512], f32, name=f"u{b}_{j}_{half}", tag="u")
                    nc.vector.tensor_scalar(
                        out=u[:], in0=pt[:],
                        scalar1=A[:, j : j + 1], scalar2=Bt[:, j : j + 1],
                        op0=mybir.AluOpType.mult, op1=mybir.AluOpType.add,
                    )
                    sg = ptiny.tile([128, 512], bf16, name=f"sg{b}_{j}_{half}", tag="sg")
                    nc.scalar.activation(sg[:], u[:], AF.Sigmoid)
                    nc.vector.tensor_mul(
                        xnT[j][:, 512 * half : 512 * (half + 1)], u[:], sg[:]
                    )

            # ---- QKV projections ----
            # q, k: [d, n] layout (4 chunks of [128, 1024] each for q and k)
            qt = []
            kt = []
            for dc in range(CC):
                t = pq.tile([128, N], bf16, name=f"q{b}_{dc}", tag="q")
                qt.append(t)
                t = pk.tile([128, N], bf16, name=f"k{b}_{dc}", tag="k")
                kt.append(t)
            for which, dst in ((0, qt), (1, kt)):
                for dc in range(CC):
                    for half in range(2):
                        pp = pps.tile(
                            [128, 512], f32, name=f"pqk{b}_{which}_{dc}_{half}",
                            tag="ps512",
                        )
                        for c in range(CC):
                            nc.tensor.matmul(
                                out=pp[:],
                                lhsT=wqkv[c][
                                    :, 512 * which + 128 * dc : 512 * which + 128 * (dc + 1)
                                ],
                                rhs=xnT[c][:, 512 * half : 512 * (half + 1)],
                                start=(c == 0), stop=(c == CC - 1),
                            )
                        nc.scalar.activation(
                            dst[dc][:, 512 * half : 512 * (half + 1)], pp[:], AF.Copy
                        )
            # v: [n, d] layout with per-head ones column: [128, 8*65] per tile
            vt = []
            for nt in range(NT):
                t = pv.tile([128, HEADS * 65], bf16, name=f"v{b}_{nt}", tag="v")
                vt.append(t)
                nc.vector.memset(
                    t[:].rearrange("p (h x) -> p h x", h=HEADS)[:, :, 64:65], 1.0
                )
                pp = pps.tile([128, 512], f32, name=f"pv{b}_{nt}", tag="ps512")
                for c in range(CC):
                    nc.tensor.matmul(
                        out=pp[:],
                        lhsT=xnT[c][:, 128 * nt : 128 * (nt + 1)],
                        rhs=wqkv[c][:, 1024:1536],
                        start=(c == 0), stop=(c == CC - 1),
                    )
                nc.vector.tensor_copy(
                    t[:].rearrange("p (h x) -> p h x", h=HEADS)[:, :, 0:64],
                    pp[:].rearrange("p (h x) -> p h x", h=HEADS),
                )

            # ---- attention per head ----
            ao = []
            for nt in range(NT):
                t = pao.tile([128, HIDDEN], bf16, name=f"ao{b}_{nt}", tag="ao")
                ao.append(t)
            for h in range(HEADS):
                dc = h // 2
                r0 = 64 * (h % 2)
                # simT[j, i] = sum_d k[d,j] q[d,i]; exp -> bf16 SBUF
                eT = []
                for jt in range(NT):
                    psim = ppsim.tile([128, N], f32, name=f"psim{b}_{h}_{jt}", tag="sim")
                    for half in range(2):
                        nc.tensor.matmul(
                            out=psim[:, 512 * half : 512 * (half + 1)],
                            lhsT=kt[dc][r0 : r0 + 64, 128 * jt : 128 * (jt + 1)],
                            rhs=qt[dc][r0 : r0 + 64, 512 * half : 512 * (half + 1)],
                        )
                    et = pe.tile([128, N], bf16, name=f"eT{b}_{h}_{jt}", tag="eT")
                    nc.scalar.activation(et[:], psim[:], AF.Exp)
                    eT.append(et)
                # PV: out[i, 0:64] = sum_j exp * v ; out[i, 64] = sumexp
                for ig in range(2):
                    ppv = pppv.tile([128, 4 * 65], f32, name=f"ppv{b}_{h}_{ig}", tag="pv")
                    for jt in range(NT):
                        for ii in range(4):
                            it = 4 * ig + ii
                            nc.tensor.matmul(
                                out=ppv[:, 65 * ii : 65 * (ii + 1)],
                                lhsT=eT[jt][:, 128 * it : 128 * (it + 1)],
                                rhs=vt[jt][:, 65 * h : 65 * (h + 1)],
                                start=(jt == 0 and ii == 0),
                                stop=(jt == NT - 1 and ii == 3),
                            )
                    for ii in range(4):
                        it = 4 * ig + ii
                        rc = prc.tile([128, 1], f32, name=f"rc{b}_{h}_{it}", tag="rc")
                        nc.vector.reciprocal(rc[:], ppv[:, 65 * ii + 64 : 65 * ii + 65])
                        nc.vector.tensor_scalar_mul(
                            ao[it][:, 64 * h : 64 * (h + 1)],
                            ppv[:, 65 * ii : 65 * ii + 64],
                            rc[:],
                        )

            # ---- transpose attn-out to [hd, n] ----
            aoT = []
            for dc in range(CC):
                t = paoT.tile([128, N], bf16, name=f"aoT{b}_{dc}", tag="aoT")
                aoT.append(t)
            for dc in range(CC):
                for half in range(2):
                    pt2 = pps.tile(
                        [128, 512], bf16, name=f"pt2{b}_{dc}_{half}", tag="ps512"
                    )
                    for q in range(4):
                        nt = 4 * half + q
                        nc.tensor.matmul(
                            out=pt2[:, 128 * q : 128 * (q + 1)],
                            lhsT=ao[nt][:, 128 * dc : 128 * (dc + 1)],
                            rhs=identb[:],
                            is_transpose=True,
                            start=(q == 0), stop=(q == 3),
                        )
                    nc.vector.tensor_copy(
                        aoT[dc][:, 512 * half : 512 * (half + 1)], pt2[:]
                    )

            # ---- out proj + residual + bias ----
            ob = pout.tile([128, NT * CH], f32, name=f"ob{b}", tag="ob")
            for nt in range(NT):
                pf = pps.tile([128, CH], f32, name=f"pf{b}_{nt}", tag="ps512")
                for dc in range(CC):
                    nc.tensor.matmul(
                        out=pf[:],
                        lhsT=aoT[dc][:, 128 * nt : 128 * (nt + 1)],
                        rhs=wout[dc][:],
                        start=(dc == 0), stop=(dc == CC - 1),
                    )
                nc.vector.tensor_add(
                    ob[:, CH * nt : CH * (nt + 1)], pf[:], xb[:, CH * nt : CH * (nt + 1)]
                )
                nc.gpsimd.tensor_add(
                    ob[:, CH * nt : CH * (nt + 1)], ob[:, CH * nt : CH * (nt + 1)], bb[:]
                )
            nc.sync.dma_start(
                out=out_d[b].rearrange("(t p) c -> p t c", p=128),
                in_=ob[:].rearrange("p (t c) -> p t c", t=NT),
            )

    nc.compile()
    return nc


def make_in_maps(x, gn_scale, gn_offset, w_qkv, w_out, b_out):
    import ml_dtypes

    bf16 = ml_dtypes.bfloat16
    x = np.asarray(x, dtype=np.float32)
    gn_scale = np.asarray(gn_scale, dtype=np.float32)
    gn_offset = np.asarray(gn_offset, dtype=np.float32)
    w_qkv = np.asarray(w_qkv, dtype=np.float32)
    w_out = np.asarray(w_out, dtype=np.float32)
    b_out = np.asarray(b_out, dtype=np.float32)

    wq = w_qkv.copy()
    wq[:, :HIDDEN] *= HEAD_CH ** -0.5  # fold q scaling
    wqkv_h = np.ascontiguousarray(wq.astype(bf16))
    wout_h = np.ascontiguousarray(w_out.astype(bf16))
    identf = np.eye(128, dtype=np.float32)
    identb = np.eye(128, dtype=np.float32).astype(bf16)
    # sel32[g, p] = 1 iff g == p // 16 (mod 8); mask32[g, j] = 1 iff g // 8 == j
    g_idx = np.arange(32)
    sel32 = (g_idx[:, None] % 8 == np.arange(128)[None, :] // 16).astype(np.float32)
    mask32 = (g_idx[:, None] // 8 == np.arange(4)[None, :]).astype(np.float32)
    # channel c = 128*j + p
    gns = np.ascontiguousarray(gn_scale.reshape(4, 128).T.astype(np.float32))
    gno = np.ascontiguousarray(gn_offset.reshape(4, 128).T.astype(np.float32))
    bb = np.broadcast_to(b_out, (128, CH)).copy()
    ones = np.ones((128, 1), dtype=np.float32)

    xr = x.reshape(B, N, CH)
    in_maps = []
    for i in range(N_CORES):
        in_maps.append(
            {
                "x": np.ascontiguousarray(xr[BPC * i : BPC * (i + 1)]),
                "wqkv": wqkv_h,
                "wout": wout_h,
                "identf": identf,
                "identb": identb,
                "sel32": sel32,
                "mask32": mask32,
                "gns": gns,
                "gno": gno,
                "bb": bb,
                "ones": ones,
            }
        )
    return in_maps


_NC_CACHE = None


def kernel(x, gn_scale, gn_offset, w_qkv, w_out, b_out, _return_extra=False):
    global _NC_CACHE
    from concourse.bass_utils import run_bass_kernel_spmd

    if _NC_CACHE is None:
        _NC_CACHE = build_program()
    nc = _NC_CACHE
    in_maps = make_in_maps(x, gn_scale, gn_offset, w_qkv, w_out, b_out)
    res = run_bass_kernel_spmd(nc, in_maps, list(range(N_CORES)))
    outs = [res.results[i]["out"] for i in range(N_CORES)]
    out = np.concatenate(outs, axis=0).reshape(B, HGT, WID, CH).astype(np.float32)
    if _return_extra:
        return out, res
    return out


# revision 4
# speedup vs baseline: 8.0063x; 8.0063x over previous
"""Trainium2 Bass kernel for nn_Attention_36146444763783.

GroupNorm(32) + SiLU -> QKV proj -> 8-head attention (n=1024) -> out proj
+ bias + residual, batch=16, fully data-parallel: 2 batches per NeuronCore
across 8 cores.

Per-core dataflow (all matmuls bf16 with fp32 PSUM accumulation):
  - x [2,1024,512] fp32 loaded as [128, 8*512] tiles (partition = token%128)
  - GroupNorm stats per (batch, group) via DVE reduces + PE ones-matmul
    partition sums; per-channel affine A,B expanded to [128,4] via a
    selector matmul so the normalize runs as one scalar-engine
    activation (Silu) per transposed block: xnT = silu(x^T * A + B)
  - QKV: q,k as [d, n] (w stationary), v as [n, d] (xnT stationary),
    with q pre-scaled by 1/8 (folded into w on host)
  - attention per head: simT[j,i] = k^T q on PE; exp on ScalarE
    (no max-subtract: |sim| < 1 for this distribution); PV accumulates
    attn-out [i, d] with an extra all-ones V column producing sumexp[i]
    on the same partitions, normalized during the PSUM drain
  - out proj from PE-transposed attn-out, residual + bias added on DVE
"""

import os
import sys

import numpy as np

sys.path.insert(0, "/opt/trn_rl_repo")

B, HGT, WID, CH = 16, 32, 32, 512
HEADS, HEAD_CH, HIDDEN = 8, 64, 512
GROUPS = 32
EPS = 1e-5
N = HGT * WID  # 1024 tokens per batch
N_CORES = 8
BPC = B // N_CORES  # batches per core
NT = N // 128  # 8 token tiles
CC = CH // 128  # 4 channel chunks


def build_program(repeat=1):
    import concourse.bacc as bacc
    import concourse.mybir as mybir
    import concourse.tile as tile
    from contextlib import ExitStack

    dt = mybir.dt
    f32, bf16 = dt.float32, dt.bfloat16
    AX = mybir.AxisListType
    AF = mybir.ActivationFunctionType

    nc = bacc.Bacc("TRN2", target_bir_lowering=False, debug=False)

    x_d = nc.dram_tensor("x", [BPC, N, CH], f32, kind="ExternalInput").ap()
    wqkv_d = nc.dram_tensor("wqkv", [CH, 3 * HIDDEN], bf16, kind="ExternalInput").ap()
    wout_d = nc.dram_tensor("wout", [HIDDEN, CH], bf16, kind="ExternalInput").ap()
    identf_d = nc.dram_tensor("identf", [128, 128], f32, kind="ExternalInput").ap()
    identb_d = nc.dram_tensor("identb", [128, 128], bf16, kind="ExternalInput").ap()
    sel32_d = nc.dram_tensor("sel32", [32, 128], f32, kind="ExternalInput").ap()
    mask32_d = nc.dram_tensor("mask32", [32, 4], f32, kind="ExternalInput").ap()
    gns_d = nc.dram_tensor("gns", [128, 4], f32, kind="ExternalInput").ap()
    gno_d = nc.dram_tensor("gno", [128, 4], f32, kind="ExternalInput").ap()
    bb_d = nc.dram_tensor("bb", [128, CH], f32, kind="ExternalInput").ap()
    ones_d = nc.dram_tensor("ones", [128, 1], f32, kind="ExternalInput").ap()
    out_d = nc.dram_tensor("out", [BPC, N, CH], f32, kind="ExternalOutput").ap()

    with ExitStack() as ctx:
        tc = ctx.enter_context(tile.TileContext(nc))
        pc = ctx.enter_context(tc.tile_pool(name="const", bufs=1))
        px = ctx.enter_context(tc.tile_pool(name="px", bufs=2))
        psq = ctx.enter_context(tc.tile_pool(name="psq", bufs=2))
        pst = ctx.enter_context(tc.tile_pool(name="pst", bufs=3))
        ptiny = ctx.enter_context(tc.tile_pool(name="ptiny", bufs=2))
        pxnT = ctx.enter_context(tc.tile_pool(name="pxnT", bufs=6))
        pq = ctx.enter_context(tc.tile_pool(name="pq", bufs=6))
        pk = ctx.enter_context(tc.tile_pool(name="pk", bufs=6))
        pv = ctx.enter_context(tc.tile_pool(name="pv", bufs=10))
        pe = ctx.enter_context(tc.tile_pool(name="pe", bufs=10))
        pao = ctx.enter_context(tc.tile_pool(name="pao", bufs=10))
        paoT = ctx.enter_context(tc.tile_pool(name="paoT", bufs=6))
        prc = ctx.enter_context(tc.tile_pool(name="prc", bufs=6))
        pout = ctx.enter_context(tc.tile_pool(name="pout", bufs=2))
        pps = ctx.enter_context(tc.tile_pool(name="pps", bufs=2, space="PSUM"))
        ppsim = ctx.enter_context(tc.tile_pool(name="ppsim", bufs=2, space="PSUM"))
        pppv = ctx.enter_context(tc.tile_pool(name="pppv", bufs=1, space="PSUM"))
        ppst = ctx.enter_context(tc.tile_pool(name="ppst", bufs=1, space="PSUM"))

        # ---- constants ----
        wqkv = []
        for j in range(CC):
            t = pc.tile([128, 3 * HIDDEN], bf16, name=f"wqkv{j}", tag=f"wqkv{j}")
            nc.sync.dma_start(out=t[:], in_=wqkv_d[128 * j : 128 * (j + 1), :])
            wqkv.append(t)
        wout = []
        for j in range(CC):
            t = pc.tile([128, CH], bf16, name=f"wout{j}", tag=f"wout{j}")
            nc.sync.dma_start(out=t[:], in_=wout_d[128 * j : 128 * (j + 1), :])
            wout.append(t)
        identf = pc.tile([128, 128], f32, name="identf", tag="identf")
        nc.sync.dma_start(out=identf[:], in_=identf_d[:, :])
        identb = pc.tile([128, 128], bf16, name="identb", tag="identb")
        nc.sync.dma_start(out=identb[:], in_=identb_d[:, :])
        sel32 = pc.tile([32, 128], f32, name="sel32", tag="sel32")
        nc.sync.dma_start(out=sel32[:], in_=sel32_d[:, :])
        mask32 = pc.tile([32, 4], f32, name="mask32", tag="mask32")
        nc.sync.dma_start(out=mask32[:], in_=mask32_d[:, :])
        gns = pc.tile([128, 4], f32, name="gns", tag="gns")
        nc.sync.dma_start(out=gns[:], in_=gns_d[:, :])
        gno = pc.tile([128, 4], f32, name="gno", tag="gno")
        nc.sync.dma_start(out=gno[:], in_=gno_d[:, :])
        bb = pc.tile([128, CH], f32, name="bb", tag="bb")
        nc.sync.dma_start(out=bb[:], in_=bb_d[:, :])
        ones = pc.tile([128, 1], f32, name="ones", tag="ones")
        nc.sync.dma_start(out=ones[:], in_=ones_d[:, :])

        for b0 in range(BPC * repeat):
            b = b0 % BPC
            # ---- load x batch: [128, 8*512], partition = token % 128 ----
            xb = px.tile([128, NT * CH], f32, name=f"xb{b0}", tag="x")
            nc.sync.dma_start(
                out=xb[:].rearrange("p (t c) -> p t c", t=NT),
                in_=x_d[b].rearrange("(t p) c -> p t c", p=128),
            )

            # ---- GroupNorm stats ----
            ps_st = ppst.tile([32, 2], f32, name=f"ps_st{b}", tag="stps")
            for nt in range(NT):
                st = pst.tile([128, 64], f32, name=f"st{b}_{nt}", tag="stats")
                xv = xb[:, CH * nt : CH * (nt + 1)].rearrange(
                    "p (g k) -> p g k", g=GROUPS
                )
                nc.vector.reduce_sum(out=st[:, 0:32], in_=xv, axis=AX.X)
                sq = psq.tile([128, CH], f32, name=f"sq{b}_{nt}", tag="sq")
                nc.vector.tensor_mul(
                    sq[:], xb[:, CH * nt : CH * (nt + 1)], xb[:, CH * nt : CH * (nt + 1)]
                )
                nc.vector.reduce_sum(
                    out=st[:, 32:64],
                    in_=sq[:].rearrange("p (g k) -> p g k", g=GROUPS),
                    axis=AX.X,
                )
                nc.tensor.matmul(
                    out=ps_st[:, 0:1], lhsT=st[:, 0:32], rhs=ones[:],
                    start=(nt == 0), stop=False,
                )
                nc.tensor.matmul(
                    out=ps_st[:, 1:2], lhsT=st[:, 32:64], rhs=ones[:],
                    start=False, stop=(nt == NT - 1),
                )

            # ---- group mean/rstd -> per-channel affine A, B [128, 4] ----
            g1 = ptiny.tile([32, 8], f32, name=f"g1{b}", tag="g1")
            inv_n = 1.0 / (N * (CH // GROUPS))
            nc.vector.tensor_scalar_mul(g1[:, 0:1], ps_st[:, 0:1], inv_n)  # mean
            nc.vector.tensor_scalar_mul(g1[:, 1:2], ps_st[:, 1:2], inv_n)  # E[x^2]
            nc.vector.tensor_mul(g1[:, 2:3], g1[:, 0:1], g1[:, 0:1])
            nc.vector.tensor_sub(g1[:, 3:4], g1[:, 1:2], g1[:, 2:3])  # var
            nc.vector.tensor_scalar_add(g1[:, 4:5], g1[:, 3:4], EPS)
            nc.vector.reciprocal(g1[:, 5:6], g1[:, 4:5])
            nc.scalar.activation(g1[:, 6:7], g1[:, 5:6], AF.Sqrt)  # rstd
            selr = ptiny.tile([32, 8], f32, name=f"selr{b}", tag="selr")
            nc.vector.tensor_scalar_mul(selr[:, 0:4], mask32[:], g1[:, 6:7])
            nc.vector.tensor_scalar_mul(selr[:, 4:8], mask32[:], g1[:, 0:1])
            ps_ab = ppst.tile([128, 8], f32, name=f"ps_ab{b}", tag="stps")
            nc.tensor.matmul(out=ps_ab[:], lhsT=sel32[:], rhs=selr[:])
            A = ptiny.tile([128, 4], f32, name=f"A{b}", tag="A")
            Bt = ptiny.tile([128, 4], f32, name=f"Bt{b}", tag="Bt")
            tmb = ptiny.tile([128, 4], f32, name=f"tmb{b}", tag="tmb")
            nc.vector.tensor_mul(A[:], ps_ab[:, 0:4], gns[:])
            nc.vector.tensor_mul(tmb[:], ps_ab[:, 4:8], A[:])
            nc.vector.tensor_sub(Bt[:], gno[:], tmb[:])

            # ---- transposed normalize: xnT[j] = silu(x^T * A + B), bf16 ----
            # silu(u) = u * sigmoid(u); u computed on DVE, sigmoid on ScalarE
            xnT = []
            for j in range(CC):
                t = pxnT.tile([128, N], bf16, name=f"xnT{b}_{j}", tag="xnT")
                xnT.append(t)
            for j in range(CC):
                for half in range(2):
                    pt = pps.tile([128, 512], f32, name=f"pt{b}_{j}_{half}", tag="ps512")
                    for q in range(4):
                        nt = 4 * half + q
                        nc.tensor.matmul(
                            out=pt[:, 128 * q : 128 * (q + 1)],
                            lhsT=xb[:, CH * nt + 128 * j : CH * nt + 128 * (j + 1)],
                            rhs=identf[:],
                            is_transpose=True,
                            start=(q == 0), stop=(q == 3),
                        )
                    u = ptiny.tile([128, 512], f32, name=f"u{b}_{j}_{half}", tag="u")
                    nc.vector.tensor_scalar(
                        out=u[:], in0=pt[:],
                        scalar1=A[:, j : j + 1], scalar2=Bt[:, j : j + 1],
                        op0=mybir.AluOpType.mult, op1=mybir.AluOpType.add,
                    )
                    sg = ptiny.tile([128, 512], bf16, name=f"sg{b}_{j}_{half}", tag="sg")
                    nc.scalar.activation(sg[:], u[:], AF.Sigmoid)
                    nc.vector.tensor_mul(
                        xnT[j][:, 512 * half : 512 * (half + 1)], u[:], sg[:]
                    )

            # ---- QKV projections ----
            # q, k: [d, n] layout (4 chunks of [128, 1024] each for q and k)
            qt = []
            kt = []
            for dc in range(CC):
                t = pq.tile([128, N], bf16, name=f"q{b}_{dc}", tag="q")
                qt.append(t)
                t = pk.tile([128, N], bf16, name=f"k{b}_{dc}", tag="k")
                kt.append(t)
            for which, dst in ((0, qt), (1, kt)):
                for dc in range(CC):
                    for half in range(2):
                        pp = pps.tile(
                            [128, 512], f32, name=f"pqk{b}_{which}_{dc}_{half}",
                            tag="ps512",
                        )
                        for c in range(CC):
                            nc.tensor.matmul(
                                out=pp[:],
                                lhsT=wqkv[c][
                                    :, 512 * which + 128 * dc : 512 * which + 128 * (dc + 1)
                                ],
                                rhs=xnT[c][:, 512 * half : 512 * (half + 1)],
                                start=(c == 0), stop=(c == CC - 1),
                            )
                        nc.scalar.activation(
                            dst[dc][:, 512 * half : 512 * (half + 1)], pp[:], AF.Copy
                        )
            # v: [n, d] layout with per-head ones column: [128, 8*65] per tile
            vt = []
            for nt in range(NT):
                t = pv.tile([128, HEADS * 65], bf16, name=f"v{b}_{nt}", tag="v")
                vt.append(t)
                nc.vector.memset(
                    t[:].rearrange("p (h x) -> p h x", h=HEADS)[:, :, 64:65], 1.0
                )
                pp = pps.tile([128, 512], f32, name=f"pv{b}_{nt}", tag="ps512")
                for c in range(CC):
                    nc.tensor.matmul(
                        out=pp[:],
                        lhsT=xnT[c][:, 128 * nt : 128 * (nt + 1)],
                        rhs=wqkv[c][:, 1024:1536],
                        start=(c == 0), stop=(c == CC - 1),
                    )
                nc.vector.tensor_copy(
                    t[:].rearrange("p (h x) -> p h x", h=HEADS)[:, :, 0:64],
                    pp[:].rearrange("p (h x) -> p h x", h=HEADS),
                )

            # ---- attention per head ----
            ao = []
            for nt in range(NT):
                t = pao.tile([128, HIDDEN], bf16, name=f"ao{b}_{nt}", tag="ao")
                ao.append(t)
            for h in range(HEADS):
                dc = h // 2
                r0 = 64 * (h % 2)
                # simT[j, i] = sum_d k[d,j] q[d,i]; exp -> bf16 SBUF
                eT = []
                for jt in range(NT):
                    psim = ppsim.tile([128, N], f32, name=f"psim{b}_{h}_{jt}", tag="sim")
                    for half in range(2):
                        nc.tensor.matmul(
                            out=psim[:, 512 * half : 512 * (half + 1)],
                            lhsT=kt[dc][r0 : r0 + 64, 128 * jt : 128 * (jt + 1)],
                            rhs=qt[dc][r0 : r0 + 64, 512 * half : 512 * (half + 1)],
                        )
                    et = pe.tile([128, N], bf16, name=f"eT{b}_{h}_{jt}", tag="eT")
                    nc.scalar.activation(et[:], psim[:], AF.Exp)
                    eT.append(et)
                # PV: out[i, 0:64] = sum_j exp * v ; out[i, 64] = sumexp
                for ig in range(2):
                    ppv = pppv.tile([128, 4 * 65], f32, name=f"ppv{b}_{h}_{ig}", tag="pv")
                    for jt in range(NT):
                        for ii in range(4):
                            it = 4 * ig + ii
                            nc.tensor.matmul(
                                out=ppv[:, 65 * ii : 65 * (ii + 1)],
                                lhsT=eT[jt][:, 128 * it : 128 * (it + 1)],
                                rhs=vt[jt][:, 65 * h : 65 * (h + 1)],
                                start=(jt == 0 and ii == 0),
                                stop=(jt == NT - 1 and ii == 3),
                            )
                    for ii in range(4):
                        it = 4 * ig + ii
                        rc = prc.tile([128, 1], f32, name=f"rc{b}_{h}_{it}", tag="rc")
                        nc.vector.reciprocal(rc[:], ppv[:, 65 * ii + 64 : 65 * ii + 65])
                        nc.vector.tensor_scalar_mul(
                            ao[it][:, 64 * h : 64 * (h + 1)],
                            ppv[:, 65 * ii : 65 * ii + 64],
                            rc[:],
                        )

            # ---- transpose attn-out to [hd, n] ----
            aoT = []
            for dc in range(CC):
                t = paoT.tile([128, N], bf16, name=f"aoT{b}_{dc}", tag="aoT")
                aoT.append(t)
            for dc in range(CC):
                for half in range(2):
                    pt2 = pps.tile(
                        [128, 512], bf16, name=f"pt2{b}_{dc}_{half}", tag="ps512"
                    )
                    for q in range(4):
                        nt = 4 * half + q
                        nc.tensor.matmul(
                            out=pt2[:, 128 * q : 128 * (q + 1)],
                            lhsT=ao[nt][:, 128 * dc : 128 * (dc + 1)],
                            rhs=identb[:],
                            is_transpose=True,
                            start=(q == 0), stop=(q == 3),
                        )
                    nc.vector.tensor_copy(
                        aoT[dc][:, 512 * half : 512 * (half + 1)], pt2[:]
                    )

            # ---- out proj + residual + bias ----
            ob = pout.tile([128, NT * CH], f32, name=f"ob{b}", tag="ob")
            for nt in range(NT):
                pf = pps.tile([128, CH], f32, name=f"pf{b}_{nt}", tag="ps512")
                for dc in range(CC):
                    nc.tensor.matmul(
                        out=pf[:],
                        lhsT=aoT[dc][:, 128 * nt : 128 * (nt + 1)],
                        rhs=wout[dc][:],
                        start=(dc == 0), stop=(dc == CC - 1),
                    )
                nc.vector.tensor_add(
                    ob[:, CH * nt : CH * (nt + 1)], pf[:], xb[:, CH * nt : CH * (nt + 1)]
                )
                nc.gpsimd.tensor_add(
                    ob[:, CH * nt : CH * (nt + 1)], ob[:, CH * nt : CH * (nt + 1)], bb[:]
                )
            nc.sync.dma_start(
                out=out_d[b].rearrange("(t p) c -> p t c", p=128),
                in_=ob[:].rearrange("p (t c) -> p t c", t=NT),
            )

    nc.compile()
    return nc


def make_in_maps(x, gn_scale, gn_offset, w_qkv, w_out, b_out):
    import ml_dtypes

    bf16 = ml_dtypes.bfloat16
    x = np.asarray(x, dtype=np.float32)
    gn_scale = np.asarray(gn_scale, dtype=np.float32)
    gn_offset = np.asarray(gn_offset, dtype=np.float32)
    w_qkv = np.asarray(w_qkv, dtype=np.float32)
    w_out = np.asarray(w_out, dtype=np.float32)
    b_out = np.asarray(b_out, dtype=np.float32)

    wq = w_qkv.copy()
    wq[:, :HIDDEN] *= HEAD_CH ** -0.5  # fold q scaling
    wqkv_h = np.ascontiguousarray(wq.astype(bf16))
    wout_h = np.ascontiguousarray(w_out.astype(bf16))
    identf = np.eye(128, dtype=np.float32)
    identb = np.eye(128, dtype=np.float32).astype(bf16)
    # sel32[g, p] = 1 iff g == p // 16 (mod 8); mask32[g, j] = 1 iff g // 8 == j
    g_idx = np.arange(32)
    sel32 = (g_idx[:, None] % 8 == np.arange(128)[None, :] // 16).astype(np.float32)
    mask32 = (g_idx[:, None] // 8 == np.arange(4)[None, :]).astype(np.float32)
    # channel c = 128*j + p
    gns = np.ascontiguousarray(gn_scale.reshape(4, 128).T.astype(np.float32))
    gno = np.ascontiguousarray(gn_offset.reshape(4, 128).T.astype(np.float32))
    bb = np.broadcast_to(b_out, (128, CH)).copy()
    ones = np.ones((128, 1), dtype=np.float32)

    xr = x.reshape(B, N, CH)
    in_maps = []
    for i in range(N_CORES):
        in_maps.append(
            {
                "x": np.ascontiguousarray(xr[BPC * i : BPC * (i + 1)]),
                "wqkv": wqkv_h,
                "wout": wout_h,
                "identf": identf,
                "identb": identb,
                "sel32": sel32,
                "mask32": mask32,
                "gns": gns,
                "gno": gno,
                "bb": bb,
                "ones": ones,
            }
        )
    return in_maps


_NC_CACHE = None


def kernel(x, gn_scale, gn_offset, w_qkv, w_out, b_out, _return_extra=False):
    global _NC_CACHE
    from concourse.bass_utils import run_bass_kernel_spmd

    if _NC_CACHE is None:
        _NC_CACHE = build_program()
    nc = _NC_CACHE
    in_maps = make_in_maps(x, gn_scale, gn_offset, w_qkv, w_out, b_out)
    res = run_bass_kernel_spmd(nc, in_maps, list(range(N_CORES)))
    outs = [res.results[i]["out"] for i in range(N_CORES)]
    out = np.concatenate(outs, axis=0).reshape(B, HGT, WID, CH).astype(np.float32)
    if _return_extra:
        return out, res
    return out
